# revision 9
# baseline (speedup 1.0000x reference)
"""Trainium2 Bass kernel for nn_AttentionConv (B=4,H=W=64,C=128,heads=2).

Sharding: 8 cores = (batch b in 0..3) x (query-half qh in 0..1).
Each core computes full attention for its 2048 query pixels of batch b,
over all 4096 keys, both heads, plus the qkv and output 1x1-conv
projections.  No cross-core communication: output row-slices are
disjoint, and the final projection contracts only over this core's own
head outputs (both heads live on the same core).

Host-side layout prep (part of sharding):
 - x[b] is transposed to [C, N] and rotated so this core's queries are
   columns 0..2047 (key order is softmax-invariant).
 - w_qkv is split into wq/wk/wv, each [128, 128] = both heads' 64 dims
   column-concatenated.

Per-core device algorithm (matmuls bf16, accumulate f32):
 - QT = wq^T x^T  -> [128(2h x 64d), 2048]   (rhs = x^T, contraction C)
 - KT = wk^T x^T  -> [128, 4096]
 - V  = x^T-chunk^T wv -> natural [4096, 128] per key-chunk as lhsT
   tiles [128, 2, 128]: cols 0..63 = v_head, cols 64..127 = 1.0 (the
   ones columns make the PV matmul also emit the softmax row-sum Z
   broadcast over partitions 64..127 of the O' psum tile).
 - per query-group (1024) x key-chunk (128): S^T = K Q^T in PSUM,
   exp via ScalarE (scale=C^-0.5 folded in), O'^T += V''^T E^T with
   the PV accumulation lagging one key-chunk behind exp.
 - normalize: resT = O'^T * recip(Z); out = resT^T w_out + b_out.

Pipeline notes: ScalarE (exp) is the bottleneck engine (~131072
elements/lane per core).  The ACT activation table is preloaded by a
dummy exp at t=0; projection outputs live in per-chunk tiles so the
attention loop starts as soon as chunk 0 is projected; the query-group
boundary is software-pipelined (next group's S^T/exp start before the
previous group's normalize + output projection).
"""

import numpy as np

import concourse.bass as bass
import concourse.tile as tile
from concourse import bacc, mybir
from concourse.bass_utils import run_bass_kernel_spmd

F32 = mybir.dt.float32
BF16 = mybir.dt.bfloat16

B = 4
C = 128
NPIX = 4096          # 64*64 pixels per batch
NQ = 2048            # queries per core (half batch)
HC = 64              # head dim
KC = 128             # key chunk
NKC = NPIX // KC     # 32
QG = 512             # query group (per head; ST tile packs both heads)
NQG = NQ // QG       # 4
SCALE = float(C) ** -0.5
N_CORES = 8

_CACHE = {}


def _build_nc():
    nc = bacc.Bacc("TRN2", target_bir_lowering=False, debug=False)

    xt_d = nc.dram_tensor("xt", [C, NPIX], F32, kind="ExternalInput")
    wq_d = nc.dram_tensor("wq", [C, 128], F32, kind="ExternalInput")
    wk_d = nc.dram_tensor("wk", [C, 128], F32, kind="ExternalInput")
    wv_d = nc.dram_tensor("wv", [C, 128], F32, kind="ExternalInput")
    wo_d = nc.dram_tensor("wo", [C, C], F32, kind="ExternalInput")
    bo_d = nc.dram_tensor("bo", [1, C], F32, kind="ExternalInput")
    out_d = nc.dram_tensor("out", [NQ, C], F32, kind="ExternalOutput")

    Exp = mybir.ActivationFunctionType.Exp

    with tile.TileContext(nc) as tc:
        with (
            tc.tile_pool(name="const", bufs=1) as const,
            tc.tile_pool(name="stage", bufs=4) as stage,
            tc.tile_pool(name="et", bufs=3) as etp,
            tc.tile_pool(name="rz", bufs=2) as rzp,
            tc.tile_pool(name="osb", bufs=2) as osbp,
            tc.tile_pool(name="pj", bufs=2, space="PSUM") as pj,
            tc.tile_pool(name="st", bufs=1, space="PSUM") as stp,
            tc.tile_pool(name="op", bufs=1, space="PSUM") as opp,
        ):
            # ---- persistent SBUF tensors (per-chunk tiles for fine deps)
            xtb = [const.tile([C, 512], BF16, tag=f"xtb{j}", name=f"xtb{j}")
                   for j in range(8)]
            qt = [const.tile([128, 512], BF16, tag=f"qt{j}", name=f"qt{j}")
                  for j in range(4)]
            kt = [const.tile([128, 512], BF16, tag=f"kt{j}", name=f"kt{j}")
                  for j in range(8)]
            v4 = [const.tile([128, 2, 128], BF16, tag=f"v4_{k}",
                             name=f"v4_{k}") for k in range(NKC)]
            rt = const.tile([128, NQ], BF16, tag="rt")
            bias_bc = const.tile([128, C], F32, tag="bias_bc")
            wqb = const.tile([C, 128], BF16, tag="wqb")
            wkb = const.tile([C, 128], BF16, tag="wkb")
            wvb = const.tile([C, 128], BF16, tag="wvb")
            wob = const.tile([C, C], BF16, tag="wob")
            ones1 = const.tile([1, C], F32, tag="ones1")
            warm = const.tile([1, 2], F32, tag="warm")

            # dummy exp first: loads the ACT table set off the critical path
            nc.vector.memset(warm[:], 0.0)
            nc.scalar.activation(warm[:], warm[:], Exp)

            # first x chunk + q/k weights first: they gate the first S^T
            xs = [stage.tile([C, 512], F32, tag="xstage", name=f"xs{j}")
                  for j in range(8)]
            nc.sync.dma_start(xs[0][:], xt_d[:, 0:512])
            for name, dram, dst in (("wq", wq_d, wqb), ("wk", wk_d, wkb),
                                    ("wv", wv_d, wvb), ("wo", wo_d, wob)):
                w32 = stage.tile([C, 128], F32, tag="w32", name=f"w32_{name}")
                nc.sync.dma_start(w32[:], dram[:])
                nc.vector.tensor_copy(dst[:], w32[:])

            # x^T load + cast + projections, chunk by chunk
            for j in range(8):
                if j > 0:
                    nc.sync.dma_start(xs[j][:], xt_d[:, j * 512:(j + 1) * 512])
                s32 = xs[j]
                nc.vector.tensor_copy(xtb[j][:], s32[:])
                if j < 4:  # QT over local queries
                    p = pj.tile([128, 512], F32, tag="pj", name=f"pq{j}")
                    nc.tensor.matmul(p[:], wqb[:], xtb[j][:],
                                     start=True, stop=True)
                    nc.vector.tensor_copy(qt[j][:], p[:])
                p = pj.tile([128, 512], F32, tag="pj", name=f"pk{j}")
                nc.tensor.matmul(p[:], wkb[:], xtb[j][:],
                                 start=True, stop=True)
                nc.vector.tensor_copy(kt[j][:], p[:])
                for kq in range(4):   # V natural per key chunk of 128
                    k = j * 4 + kq
                    pv = pj.tile([128, 128], F32, tag="pj", name=f"pv{k}")
                    nc.tensor.matmul(pv[:],
                                     xtb[j][:, kq * 128:(kq + 1) * 128],
                                     wvb[:], start=True, stop=True)
                    nc.vector.memset(v4[k][:, :, 64:128], 1.0)
                    nc.vector.tensor_copy(
                        v4[k][:, :, 0:64],
                        pv[:].rearrange("p (h d) -> p h d", h=2))

            # bias broadcast to all partitions via ones-matmul (only
            # needed by the output projection, so emitted late)
            bo32 = stage.tile([1, C], F32, tag="bo32")
            nc.sync.dma_start(bo32[:], bo_d[:])
            nc.vector.memset(ones1[:], 1.0)
            bps = pj.tile([128, C], F32, tag="pj", name="bps")
            nc.tensor.matmul(bps[:], ones1[:], bo32[:], start=True, stop=True)
            nc.vector.tensor_copy(bias_bc[:], bps[:])

            # ---- attention (software-pipelined across query groups) ----
            # ST tile [128, 1024] packs h0 queries in [:, 0:512] and h1
            # queries in [:, 512:1024]: the two S^T matmuls hit disjoint
            # row-groups (K=64 at base partition 0 / 64) and disjoint
            # PSUM banks, so they run concurrently on the PE array, and
            # one ACT instruction exps both heads.
            def emit_finalize(qg, o_ps):
                q0 = qg * QG
                for h in range(2):
                    rz = rzp.tile([64, QG], F32, tag=f"rz{h}",
                                  name=f"rz{h}_{qg}")
                    nc.vector.reciprocal(rz[:], o_ps[h][64:128, :])
                    nc.vector.tensor_mul(
                        rt[h * HC:(h + 1) * HC, q0:q0 + QG],
                        o_ps[h][0:64, :], rz[:])
                # output projection, batched 4 pixel-chunks per psum bank
                gp = opp.tile([128, 512], F32, tag=f"o{qg % 2}",
                              name=f"gps_{qg}")
                for i in range(4):
                    nc.tensor.matmul(
                        gp[:, i * 128:(i + 1) * 128],
                        rt[:, q0 + i * 128:q0 + (i + 1) * 128],
                        wob[:], start=True, stop=True)
                ob = osbp.tile([128, 512], F32, tag="osb", name=f"ob_{qg}")
                for i in range(4):
                    nc.vector.tensor_add(
                        ob[:, i * 128:(i + 1) * 128],
                        gp[:, i * 128:(i + 1) * 128], bias_bc[:])
                nc.sync.dma_start(
                    out_d[q0:q0 + QG, :].rearrange("(c r) w -> r c w", r=128),
                    ob[:].rearrange("p (c w) -> p c w", w=128))

            pending = None     # (qg, o_ps) awaiting finalize
            for qg in range(NQG):
                o_ps = [opp.tile([128, QG], F32, tag=f"o{h}",
                                 name=f"o_ps{h}_{qg}") for h in range(2)]
                prev_et = None
                for kc in range(NKC + 1):
                    if kc < NKC:
                        st = stp.tile([128, 2 * QG], F32, tag="st",
                                      bufs=2, name=f"st_{qg}_{kc}")
                        ktt = kt[kc // 4]
                        ks = slice((kc % 4) * 128, (kc % 4 + 1) * 128)
                        for h in range(2):
                            hp = slice(h * HC, (h + 1) * HC)
                            nc.tensor.matmul(
                                st[:, h * QG:(h + 1) * QG],
                                ktt[hp, ks], qt[qg][hp, :],
                                start=True, stop=True)
                    if kc > 0:
                        pk = kc - 1
                        for h in range(2):
                            nc.tensor.matmul(
                                o_ps[h][:], v4[pk][:, h, :],
                                prev_et[:, h * QG:(h + 1) * QG],
                                start=(pk == 0), stop=(pk == NKC - 1))
                    if kc < NKC:
                        et = etp.tile([128, 2 * QG], BF16, tag="et",
                                      bufs=4, name=f"et_{qg}_{kc}")
                        nc.scalar.activation(et[:], st[:], Exp, scale=SCALE)
                        prev_et = et
                    if kc == 2 and pending is not None:
                        emit_finalize(*pending)
                        pending = None
                pending = (qg, o_ps)
            emit_finalize(*pending)

    nc.compile()
    return nc


def _prep_in_maps(x, w_qkv, w_out, b_out):
    x = np.asarray(x, dtype=np.float32).reshape(B, NPIX, C)
    w_qkv = np.asarray(w_qkv, dtype=np.float32)
    w_out = np.asarray(w_out, dtype=np.float32)
    b_out = np.asarray(b_out, dtype=np.float32)

    wq = np.ascontiguousarray(
        np.concatenate([w_qkv[:, 0:64], w_qkv[:, 192:256]], axis=1))
    wk = np.ascontiguousarray(
        np.concatenate([w_qkv[:, 64:128], w_qkv[:, 256:320]], axis=1))
    wv = np.ascontiguousarray(
        np.concatenate([w_qkv[:, 128:192], w_qkv[:, 320:384]], axis=1))
    wo = np.ascontiguousarray(w_out)
    bo = np.ascontiguousarray(b_out.reshape(1, C))

    in_maps = []
    for core in range(N_CORES):
        b, qh = core // 2, core % 2
        xbT = x[b].T                     # [C, NPIX]
        q0 = qh * NQ
        xt = np.ascontiguousarray(
            np.concatenate([xbT[:, q0:], xbT[:, :q0]], axis=1))
        in_maps.append({"xt": xt, "wq": wq, "wk": wk, "wv": wv,
                        "wo": wo, "bo": bo})
    return in_maps


def run(x, w_qkv, w_out, b_out, trace=False, **run_kwargs):
    if "nc" not in _CACHE:
        _CACHE["nc"] = _build_nc()
    nc = _CACHE["nc"]
    in_maps = _prep_in_maps(x, w_qkv, w_out, b_out)
    res = run_bass_kernel_spmd(nc, in_maps, core_ids=list(range(N_CORES)),
                               trace=trace, **run_kwargs)
    out = np.empty((B, NPIX, C), dtype=np.float32)
    for core in range(N_CORES):
        b, qh = core // 2, core % 2
        out[b, qh * NQ:(qh + 1) * NQ, :] = res.results[core]["out"]
    return out.reshape(B, 64, 64, C), res


def kernel(x, w_qkv, w_out, b_out):
    out, _ = run(x, w_qkv, w_out, b_out, trace=False)
    return out


# revision 12
# speedup vs baseline: 1.0184x; 1.0184x over previous
"""Trainium2 Bass kernel for nn_AttentionConv (B=4,H=W=64,C=128,heads=2).

Sharding: 8 cores = (batch b in 0..3) x (query-half qh in 0..1).
Each core computes full attention for its 2048 query pixels of batch b,
over all 4096 keys, both heads, plus the qkv and output 1x1-conv
projections.  No cross-core communication: output row-slices are
disjoint, and the final projection contracts only over this core's own
head outputs (both heads live on the same core).

Host-side layout prep (part of sharding):
 - x[b] is transposed to [C, N] and rotated so this core's queries are
   columns 0..2047 (key order is softmax-invariant).
 - w_qkv is split into wq/wk/wv, each [128, 128] = both heads' 64 dims
   column-concatenated.

Per-core device algorithm (matmuls bf16, accumulate f32):
 - QT = wq^T x^T  -> [128(2h x 64d), 2048]   (rhs = x^T, contraction C)
 - KT = wk^T x^T  -> [128, 4096]
 - V  = x^T-chunk^T wv -> natural [4096, 128] per key-chunk as lhsT
   tiles [128, 2, 128]: cols 0..63 = v_head, cols 64..127 = 1.0 (the
   ones columns make the PV matmul also emit the softmax row-sum Z
   broadcast over partitions 64..127 of the O' psum tile).
 - per query-group (1024) x key-chunk (128): S^T = K Q^T in PSUM,
   exp via ScalarE (scale=C^-0.5 folded in), O'^T += V''^T E^T with
   the PV accumulation lagging one key-chunk behind exp.
 - normalize: resT = O'^T * recip(Z); out = resT^T w_out + b_out.

Pipeline notes: ScalarE (exp) is the bottleneck engine (~131072
elements/lane per core).  The ACT activation table is preloaded by a
dummy exp at t=0; projection outputs live in per-chunk tiles so the
attention loop starts as soon as chunk 0 is projected; the query-group
boundary is software-pipelined (next group's S^T/exp start before the
previous group's normalize + output projection).
"""

import numpy as np

import concourse.bass as bass
import concourse.tile as tile
from concourse import bacc, mybir
from concourse.bass_utils import run_bass_kernel_spmd

F32 = mybir.dt.float32
BF16 = mybir.dt.bfloat16

B = 4
C = 128
NPIX = 4096          # 64*64 pixels per batch
NQ = 2048            # queries per core (half batch)
HC = 64              # head dim
KC = 128             # key chunk
NKC = NPIX // KC     # 32
QG = 512             # query group (per head; ST tile packs both heads)
NQG = NQ // QG       # 4
SCALE = float(C) ** -0.5
N_CORES = 8

_CACHE = {}


def _build_nc():
    nc = bacc.Bacc("TRN2", target_bir_lowering=False, debug=False)

    xt_d = nc.dram_tensor("xt", [C, NPIX], F32, kind="ExternalInput")
    wq_d = nc.dram_tensor("wq", [C, 128], F32, kind="ExternalInput")
    wk_d = nc.dram_tensor("wk", [C, 128], F32, kind="ExternalInput")
    wv_d = nc.dram_tensor("wv", [C, 128], F32, kind="ExternalInput")
    wo_d = nc.dram_tensor("wo", [C, C], F32, kind="ExternalInput")
    bo_d = nc.dram_tensor("bo", [1, C], F32, kind="ExternalInput")
    out_d = nc.dram_tensor("out", [NQ, C], F32, kind="ExternalOutput")

    Exp = mybir.ActivationFunctionType.Exp

    with tile.TileContext(nc) as tc:
        with (
            tc.tile_pool(name="const", bufs=1) as const,
            tc.tile_pool(name="stage", bufs=4) as stage,
            tc.tile_pool(name="et", bufs=3) as etp,
            tc.tile_pool(name="rz", bufs=2) as rzp,
            tc.tile_pool(name="osb", bufs=2) as osbp,
            tc.tile_pool(name="st", bufs=1, space="PSUM") as stp,
            tc.tile_pool(name="op", bufs=1, space="PSUM") as opp,
        ):
            # ---- persistent SBUF tensors (per-chunk tiles for fine deps)
            xtb = [const.tile([C, 512], BF16, tag=f"xtb{j}", name=f"xtb{j}")
                   for j in range(8)]
            qt = [const.tile([128, 512], BF16, tag=f"qt{j}", name=f"qt{j}")
                  for j in range(4)]
            kt = [const.tile([128, 512], BF16, tag=f"kt{j}", name=f"kt{j}")
                  for j in range(8)]
            v4 = [const.tile([128, 2, 128], BF16, tag=f"v4_{k}",
                             name=f"v4_{k}") for k in range(NKC)]
            rt = const.tile([128, NQ], BF16, tag="rt")
            bias_bc = const.tile([128, C], F32, tag="bias_bc")
            wqb = const.tile([C, 128], BF16, tag="wqb")
            wkb = const.tile([C, 128], BF16, tag="wkb")
            wvb = const.tile([C, 128], BF16, tag="wvb")
            wob = const.tile([C, C], BF16, tag="wob")
            ones1 = const.tile([1, C], F32, tag="ones1")
            warm = const.tile([1, 2], F32, tag="warm")

            # dummy exp first: loads the ACT table set off the critical path
            nc.vector.memset(warm[:], 0.0)
            nc.scalar.activation(warm[:], warm[:], Exp)

            # PE warm-up: ~4us of dummy matmuls while DMAs run, so the
            # HAM clock-gate reaches K=8/8 before the real matmuls start
            junk = const.tile([C, 512], BF16, tag="junk")
            nc.vector.memset(junk[:], 0.25)
            wst = stp.tile([128, 2 * QG], F32, tag="st", bufs=2, name="warm_st")
            for w in range(10):
                nc.tensor.matmul(wst[:, 0:512], junk[:, 0:128], junk[:],
                                 start=True, stop=True)

            # first x chunk + q/k weights first: they gate the first S^T
            xs = [stage.tile([C, 512], F32, tag="xstage", name=f"xs{j}")
                  for j in range(8)]
            nc.sync.dma_start(xs[0][:], xt_d[:, 0:512])
            for name, dram, dst in (("wq", wq_d, wqb), ("wk", wk_d, wkb),
                                    ("wv", wv_d, wvb), ("wo", wo_d, wob)):
                w32 = stage.tile([C, 128], F32, tag="w32", name=f"w32_{name}")
                nc.sync.dma_start(w32[:], dram[:])
                nc.vector.tensor_copy(dst[:], w32[:])

            # x^T load + cast + projections, chunk by chunk
            for j in range(8):
                if j > 0:
                    nc.sync.dma_start(xs[j][:], xt_d[:, j * 512:(j + 1) * 512])
                s32 = xs[j]
                nc.vector.tensor_copy(xtb[j][:], s32[:])
                if j < 4:  # QT over local queries
                    p = opp.tile([128, 512], F32, tag="o2", name=f"pq{j}")
                    nc.tensor.matmul(p[:], wqb[:], xtb[j][:],
                                     start=True, stop=True)
                    nc.vector.tensor_copy(qt[j][:], p[:])
                p = opp.tile([128, 512], F32, tag="o3", name=f"pk{j}")
                nc.tensor.matmul(p[:], wkb[:], xtb[j][:],
                                 start=True, stop=True)
                nc.vector.tensor_copy(kt[j][:], p[:])
                for kq in range(4):   # V natural per key chunk of 128
                    k = j * 4 + kq
                    pv = opp.tile([128, 128], F32, tag=f"o{2 + k % 2}",
                                  name=f"pv{k}")
                    nc.tensor.matmul(pv[:],
                                     xtb[j][:, kq * 128:(kq + 1) * 128],
                                     wvb[:], start=True, stop=True)
                    nc.vector.memset(v4[k][:, :, 64:128], 1.0)
                    nc.vector.tensor_copy(
                        v4[k][:, :, 0:64],
                        pv[:].rearrange("p (h d) -> p h d", h=2))

            # bias broadcast to all partitions via ones-matmul (only
            # needed by the output projection, so emitted late)
            bo32 = stage.tile([1, C], F32, tag="bo32")
            nc.sync.dma_start(bo32[:], bo_d[:])
            nc.vector.memset(ones1[:], 1.0)
            bps = opp.tile([128, C], F32, tag="o2", name="bps")
            nc.tensor.matmul(bps[:], ones1[:], bo32[:], start=True, stop=True)
            nc.vector.tensor_copy(bias_bc[:], bps[:])

            # ---- attention (software-pipelined across query groups) ----
            # ST tile [128, 1024] packs h0 queries in [:, 0:512] and h1
            # queries in [:, 512:1024]: the two S^T matmuls hit disjoint
            # row-groups (K=64 at base partition 0 / 64) and disjoint
            # PSUM banks, so they run concurrently on the PE array, and
            # one ACT instruction exps both heads.
            def emit_norm(qg, o_ps):
                # DVE-only: recip(Z) + scale, emitted early so it runs
                # while the next group's attention keeps PE/ACT busy
                q0 = qg * QG
                for h in range(2):
                    rz = rzp.tile([64, QG], F32, tag=f"rz{h}",
                                  name=f"rz{h}_{qg}")
                    nc.vector.reciprocal(rz[:], o_ps[h][64:128, :])
                    nc.vector.tensor_mul(
                        rt[h * HC:(h + 1) * HC, q0:q0 + QG],
                        o_ps[h][0:64, :], rz[:])

            def emit_outproj(qg):
                # PE part of the epilogue, emitted ~8 key-chunks later so
                # the normalize has finished by the time PE reaches it
                q0 = qg * QG
                gp = opp.tile([128, 512], F32, tag=f"o{(2 * qg) % 4}",
                              name=f"gps_{qg}")
                for i in range(4):
                    nc.tensor.matmul(
                        gp[:, i * 128:(i + 1) * 128],
                        rt[:, q0 + i * 128:q0 + (i + 1) * 128],
                        wob[:], start=True, stop=True)
                ob = osbp.tile([128, 512], F32, tag="osb", name=f"ob_{qg}")
                for i in range(4):
                    nc.vector.tensor_add(
                        ob[:, i * 128:(i + 1) * 128],
                        gp[:, i * 128:(i + 1) * 128], bias_bc[:])
                nc.sync.dma_start(
                    out_d[q0:q0 + QG, :].rearrange("(c r) w -> r c w", r=128),
                    ob[:].rearrange("p (c w) -> p c w", w=128))

            pending = None     # (qg, o_ps) awaiting finalize
            for qg in range(NQG):
                o_ps = [opp.tile([128, QG], F32,
                                 tag=f"o{(2 * qg + h) % 4}",
                                 name=f"o_ps{h}_{qg}") for h in range(2)]
                prev_et = None
                for kc in range(NKC + 1):
                    if kc < NKC:
                        st = stp.tile([128, 2 * QG], F32, tag="st",
                                      bufs=2, name=f"st_{qg}_{kc}")
                        ktt = kt[kc // 4]
                        ks = slice((kc % 4) * 128, (kc % 4 + 1) * 128)
                        for h in range(2):
                            hp = slice(h * HC, (h + 1) * HC)
                            nc.tensor.matmul(
                                st[:, h * QG:(h + 1) * QG],
                                ktt[hp, ks], qt[qg][hp, :],
                                start=True, stop=True)
                    if kc > 0:
                        pk = kc - 1
                        for h in range(2):
                            nc.tensor.matmul(
                                o_ps[h][:], v4[pk][:, h, :],
                                prev_et[:, h * QG:(h + 1) * QG],
                                start=(pk == 0), stop=(pk == NKC - 1))
                    if kc < NKC:
                        et = etp.tile([128, 2 * QG], BF16, tag="et",
                                      bufs=4, name=f"et_{qg}_{kc}")
                        nc.scalar.activation(et[:], st[:], Exp, scale=SCALE)
                        prev_et = et
                    if kc == 2 and pending is not None:
                        emit_norm(*pending)
                    if kc == 10 and pending is not None:
                        emit_outproj(pending[0])
                        pending = None
                pending = (qg, o_ps)
            emit_norm(*pending)
            emit_outproj(pending[0])

    nc.compile()
    return nc


def _prep_in_maps(x, w_qkv, w_out, b_out):
    x = np.asarray(x, dtype=np.float32).reshape(B, NPIX, C)
    w_qkv = np.asarray(w_qkv, dtype=np.float32)
    w_out = np.asarray(w_out, dtype=np.float32)
    b_out = np.asarray(b_out, dtype=np.float32)

    wq = np.ascontiguousarray(
        np.concatenate([w_qkv[:, 0:64], w_qkv[:, 192:256]], axis=1))
    wk = np.ascontiguousarray(
        np.concatenate([w_qkv[:, 64:128], w_qkv[:, 256:320]], axis=1))
    wv = np.ascontiguousarray(
        np.concatenate([w_qkv[:, 128:192], w_qkv[:, 320:384]], axis=1))
    wo = np.ascontiguousarray(w_out)
    bo = np.ascontiguousarray(b_out.reshape(1, C))

    in_maps = []
    for core in range(N_CORES):
        b, qh = core // 2, core % 2
        xbT = x[b].T                     # [C, NPIX]
        q0 = qh * NQ
        xt = np.ascontiguousarray(
            np.concatenate([xbT[:, q0:], xbT[:, :q0]], axis=1))
        in_maps.append({"xt": xt, "wq": wq, "wk": wk, "wv": wv,
                        "wo": wo, "bo": bo})
    return in_maps


def run(x, w_qkv, w_out, b_out, trace=False, **run_kwargs):
    if "nc" not in _CACHE:
        _CACHE["nc"] = _build_nc()
    nc = _CACHE["nc"]
    in_maps = _prep_in_maps(x, w_qkv, w_out, b_out)
    res = run_bass_kernel_spmd(nc, in_maps, core_ids=list(range(N_CORES)),
                               trace=trace, **run_kwargs)
    out = np.empty((B, NPIX, C), dtype=np.float32)
    for core in range(N_CORES):
        b, qh = core // 2, core % 2
        out[b, qh * NQ:(qh + 1) * NQ, :] = res.results[core]["out"]
    return out.reshape(B, 64, 64, C), res


def kernel(x, w_qkv, w_out, b_out):
    out, _ = run(x, w_qkv, w_out, b_out, trace=False)
    return out


# revision 13
# speedup vs baseline: 1.1189x; 1.0986x over previous
"""Trainium2 Bass kernel for nn_AttentionConv (B=4,H=W=64,C=128,heads=2).

Sharding: 8 cores = (batch b in 0..3) x (query-half qh in 0..1).
Each core computes full attention for its 2048 query pixels of batch b,
over all 4096 keys, both heads, plus the qkv and output 1x1-conv
projections.  No cross-core communication: output row-slices are
disjoint, and the final projection contracts only over this core's own
head outputs (both heads live on the same core).

Host-side layout prep (part of sharding):
 - x[b] is transposed to [C, N] and rotated so this core's queries are
   columns 0..2047 (key order is softmax-invariant).
 - w_qkv is split into wq/wk/wv, each [128, 128] = both heads' 64 dims
   column-concatenated.

Per-core device algorithm (matmuls bf16, accumulate f32):
 - QT = wq^T x^T  -> [128(2h x 64d), 2048]   (rhs = x^T, contraction C)
 - KT = wk^T x^T  -> [128, 4096]
 - V  = x^T-chunk^T wv -> natural [4096, 128] per key-chunk as lhsT
   tiles [128, 2, 128]: cols 0..63 = v_head, cols 64..127 = 1.0 (the
   ones columns make the PV matmul also emit the softmax row-sum Z
   broadcast over partitions 64..127 of the O' psum tile).
 - per query-group (1024) x key-chunk (128): S^T = K Q^T in PSUM,
   exp via ScalarE (scale=C^-0.5 folded in), O'^T += V''^T E^T with
   the PV accumulation lagging one key-chunk behind exp.
 - normalize: resT = O'^T * recip(Z); out = resT^T w_out + b_out.

Pipeline notes: ScalarE (exp) is the bottleneck engine (~131072
elements/lane per core).  The ACT activation table is preloaded by a
dummy exp at t=0; projection outputs live in per-chunk tiles so the
attention loop starts as soon as chunk 0 is projected; the query-group
boundary is software-pipelined (next group's S^T/exp start before the
previous group's normalize + output projection).
"""

import numpy as np

import concourse.bass as bass
import concourse.tile as tile
from concourse.tile import add_dep_helper
from concourse import bacc, mybir
from concourse.bass_utils import run_bass_kernel_spmd

F32 = mybir.dt.float32
BF16 = mybir.dt.bfloat16

B = 4
C = 128
NPIX = 4096          # 64*64 pixels per batch
NQ = 2048            # queries per core (half batch)
HC = 64              # head dim
KC = 128             # key chunk
NKC = NPIX // KC     # 32
QG = 512             # query group (per head; ST tile packs both heads)
NQG = NQ // QG       # 4
SCALE = float(C) ** -0.5
N_CORES = 8

_CACHE = {}


def _build_nc():
    nc = bacc.Bacc("TRN2", target_bir_lowering=False, debug=False)

    xt_d = nc.dram_tensor("xt", [C, NPIX], F32, kind="ExternalInput")
    wq_d = nc.dram_tensor("wq", [C, 128], F32, kind="ExternalInput")
    wk_d = nc.dram_tensor("wk", [C, 128], F32, kind="ExternalInput")
    wv_d = nc.dram_tensor("wv", [C, 128], F32, kind="ExternalInput")
    wo_d = nc.dram_tensor("wo", [C, C], F32, kind="ExternalInput")
    bo_d = nc.dram_tensor("bo", [1, C], F32, kind="ExternalInput")
    out_d = nc.dram_tensor("out", [NQ, C], F32, kind="ExternalOutput")

    Exp = mybir.ActivationFunctionType.Exp

    with tile.TileContext(nc) as tc:
        with (
            tc.tile_pool(name="const", bufs=1) as const,
            tc.tile_pool(name="stage", bufs=4) as stage,
            tc.tile_pool(name="et", bufs=3) as etp,
            tc.tile_pool(name="rz", bufs=2) as rzp,
            tc.tile_pool(name="osb", bufs=2) as osbp,
            tc.tile_pool(name="st", bufs=1, space="PSUM") as stp,
            tc.tile_pool(name="op", bufs=1, space="PSUM") as opp,
        ):
            # ---- persistent SBUF tensors (per-chunk tiles for fine deps)
            xtb = [const.tile([C, 512], BF16, tag=f"xtb{j}", name=f"xtb{j}")
                   for j in range(8)]
            qt = [const.tile([128, 512], BF16, tag=f"qt{j}", name=f"qt{j}")
                  for j in range(4)]
            kt = [const.tile([128, 512], BF16, tag=f"kt{j}", name=f"kt{j}")
                  for j in range(8)]
            v4 = [const.tile([128, 2, 128], BF16, tag=f"v4_{k}",
                             name=f"v4_{k}") for k in range(NKC)]
            rt = const.tile([128, NQ], BF16, tag="rt")
            bias_bc = const.tile([128, C], F32, tag="bias_bc")
            wqb = const.tile([C, 128], BF16, tag="wqb")
            wkb = const.tile([C, 128], BF16, tag="wkb")
            wvb = const.tile([C, 128], BF16, tag="wvb")
            wob = const.tile([C, C], BF16, tag="wob")
            ones1 = const.tile([1, C], F32, tag="ones1")
            warm = const.tile([1, 2], F32, tag="warm")

            # dummy exp first: loads the ACT table set off the critical path
            nc.vector.memset(warm[:], 0.0)
            nc.scalar.activation(warm[:], warm[:], Exp)

            # PE warm-up: ~4us of dummy matmuls while DMAs run, so the
            # HAM clock-gate reaches K=8/8 before the real matmuls start
            junk = const.tile([C, 512], BF16, tag="junk")
            nc.vector.memset(junk[:], 0.25)
            wst = stp.tile([128, 2 * QG], F32, tag="st", bufs=2, name="warm_st")
            for w in range(10):
                nc.tensor.matmul(wst[:, 0:512], junk[:, 0:128], junk[:],
                                 start=True, stop=True)

            # first x chunk + q/k weights first: they gate the first S^T
            xs = [stage.tile([C, 512], F32, tag="xstage", name=f"xs{j}")
                  for j in range(8)]
            nc.sync.dma_start(xs[0][:], xt_d[:, 0:512])
            for name, dram, dst in (("wq", wq_d, wqb), ("wk", wk_d, wkb),
                                    ("wv", wv_d, wvb), ("wo", wo_d, wob)):
                w32 = stage.tile([C, 128], F32, tag="w32", name=f"w32_{name}")
                nc.sync.dma_start(w32[:], dram[:])
                nc.vector.tensor_copy(dst[:], w32[:])

            # x^T load + cast + projections, chunk by chunk
            for j in range(8):
                if j > 0:
                    nc.sync.dma_start(xs[j][:], xt_d[:, j * 512:(j + 1) * 512])
                s32 = xs[j]
                nc.vector.tensor_copy(xtb[j][:], s32[:])
                if j < 4:  # QT over local queries
                    p = opp.tile([128, 512], F32, tag="o2", name=f"pq{j}")
                    nc.tensor.matmul(p[:], wqb[:], xtb[j][:],
                                     start=True, stop=True)
                    nc.vector.tensor_copy(qt[j][:], p[:])
                p = opp.tile([128, 512], F32, tag="o3", name=f"pk{j}")
                nc.tensor.matmul(p[:], wkb[:], xtb[j][:],
                                 start=True, stop=True)
                nc.vector.tensor_copy(kt[j][:], p[:])
                for kq in range(4):   # V natural per key chunk of 128
                    k = j * 4 + kq
                    pv = opp.tile([128, 128], F32, tag=f"o{2 + k % 2}",
                                  name=f"pv{k}")
                    nc.tensor.matmul(pv[:],
                                     xtb[j][:, kq * 128:(kq + 1) * 128],
                                     wvb[:], start=True, stop=True)
                    nc.vector.memset(v4[k][:, :, 64:128], 1.0)
                    nc.vector.tensor_copy(
                        v4[k][:, :, 0:64],
                        pv[:].rearrange("p (h d) -> p h d", h=2))

            # bias broadcast to all partitions via ones-matmul (only
            # needed by the output projection, so emitted late)
            bo32 = stage.tile([1, C], F32, tag="bo32")
            nc.sync.dma_start(bo32[:], bo_d[:])
            nc.vector.memset(ones1[:], 1.0)
            bps = opp.tile([128, C], F32, tag="o2", name="bps")
            nc.tensor.matmul(bps[:], ones1[:], bo32[:], start=True, stop=True)
            nc.vector.tensor_copy(bias_bc[:], bps[:])

            # ---- attention (software-pipelined across query groups) ----
            # ST tile [128, 1024] packs h0 queries in [:, 0:512] and h1
            # queries in [:, 512:1024]: the two S^T matmuls hit disjoint
            # row-groups (K=64 at base partition 0 / 64) and disjoint
            # PSUM banks, so they run concurrently on the PE array, and
            # one ACT instruction exps both heads.
            def emit_norm(qg, o_ps):
                # DVE-only: recip(Z) + scale, emitted early so it runs
                # while the next group's attention keeps PE/ACT busy
                q0 = qg * QG
                for h in range(2):
                    rz = rzp.tile([64, QG], F32, tag=f"rz{h}",
                                  name=f"rz{h}_{qg}")
                    nc.vector.reciprocal(rz[:], o_ps[h][64:128, :])
                    nc.vector.tensor_mul(
                        rt[h * HC:(h + 1) * HC, q0:q0 + QG],
                        o_ps[h][0:64, :], rz[:])

            def emit_outproj(qg, anchor):
                # PE part of the epilogue, emitted ~8 key-chunks later so
                # the normalize has finished by the time PE reaches it
                q0 = qg * QG
                gp = opp.tile([128, 512], F32, tag=f"o{(2 * qg) % 4}",
                              name=f"gps_{qg}")
                for i in range(4):
                    mm = nc.tensor.matmul(
                        gp[:, i * 128:(i + 1) * 128],
                        rt[:, q0 + i * 128:q0 + (i + 1) * 128],
                        wob[:], start=True, stop=True)
                    if anchor is not None:
                        add_dep_helper(mm.ins, anchor.ins, False,
                                       "outproj after next-qg S^T kc8")
                ob = osbp.tile([128, 512], F32, tag="osb", name=f"ob_{qg}")
                for i in range(4):
                    nc.vector.tensor_add(
                        ob[:, i * 128:(i + 1) * 128],
                        gp[:, i * 128:(i + 1) * 128], bias_bc[:])
                nc.sync.dma_start(
                    out_d[q0:q0 + QG, :].rearrange("(c r) w -> r c w", r=128),
                    ob[:].rearrange("p (c w) -> p c w", w=128))

            pending = None     # (qg, o_ps) awaiting finalize
            for qg in range(NQG):
                o_ps = [opp.tile([128, QG], F32,
                                 tag=f"o{(2 * qg + h) % 4}",
                                 name=f"o_ps{h}_{qg}") for h in range(2)]
                prev_et = None
                anchor_mm = None
                for kc in range(NKC + 1):
                    if kc < NKC:
                        st = stp.tile([128, 2 * QG], F32, tag="st",
                                      bufs=2, name=f"st_{qg}_{kc}")
                        ktt = kt[kc // 4]
                        ks = slice((kc % 4) * 128, (kc % 4 + 1) * 128)
                        for h in range(2):
                            hp = slice(h * HC, (h + 1) * HC)
                            mm = nc.tensor.matmul(
                                st[:, h * QG:(h + 1) * QG],
                                ktt[hp, ks], qt[qg][hp, :],
                                start=True, stop=True)
                            if kc == 8 and h == 0:
                                anchor_mm = mm
                    if kc > 0:
                        pk = kc - 1
                        for h in range(2):
                            nc.tensor.matmul(
                                o_ps[h][:], v4[pk][:, h, :],
                                prev_et[:, h * QG:(h + 1) * QG],
                                start=(pk == 0), stop=(pk == NKC - 1))
                    if kc < NKC:
                        et = etp.tile([128, 2 * QG], BF16, tag="et",
                                      bufs=4, name=f"et_{qg}_{kc}")
                        nc.scalar.activation(et[:], st[:], Exp, scale=SCALE)
                        prev_et = et
                    if kc == 2 and pending is not None:
                        emit_norm(*pending)
                    if kc == 10 and pending is not None:
                        emit_outproj(pending[0], anchor_mm)
                        pending = None
                pending = (qg, o_ps)
            emit_norm(*pending)
            emit_outproj(pending[0], None)

    nc.compile()
    return nc


def _prep_in_maps(x, w_qkv, w_out, b_out):
    x = np.asarray(x, dtype=np.float32).reshape(B, NPIX, C)
    w_qkv = np.asarray(w_qkv, dtype=np.float32)
    w_out = np.asarray(w_out, dtype=np.float32)
    b_out = np.asarray(b_out, dtype=np.float32)

    wq = np.ascontiguousarray(
        np.concatenate([w_qkv[:, 0:64], w_qkv[:, 192:256]], axis=1))
    wk = np.ascontiguousarray(
        np.concatenate([w_qkv[:, 64:128], w_qkv[:, 256:320]], axis=1))
    wv = np.ascontiguousarray(
        np.concatenate([w_qkv[:, 128:192], w_qkv[:, 320:384]], axis=1))
    wo = np.ascontiguousarray(w_out)
    bo = np.ascontiguousarray(b_out.reshape(1, C))

    in_maps = []
    for core in range(N_CORES):
        b, qh = core // 2, core % 2
        xbT = x[b].T                     # [C, NPIX]
        q0 = qh * NQ
        xt = np.ascontiguousarray(
            np.concatenate([xbT[:, q0:], xbT[:, :q0]], axis=1))
        in_maps.append({"xt": xt, "wq": wq, "wk": wk, "wv": wv,
                        "wo": wo, "bo": bo})
    return in_maps


def run(x, w_qkv, w_out, b_out, trace=False, **run_kwargs):
    if "nc" not in _CACHE:
        _CACHE["nc"] = _build_nc()
    nc = _CACHE["nc"]
    in_maps = _prep_in_maps(x, w_qkv, w_out, b_out)
    res = run_bass_kernel_spmd(nc, in_maps, core_ids=list(range(N_CORES)),
                               trace=trace, **run_kwargs)
    out = np.empty((B, NPIX, C), dtype=np.float32)
    for core in range(N_CORES):
        b, qh = core // 2, core % 2
        out[b, qh * NQ:(qh + 1) * NQ, :] = res.results[core]["out"]
    return out.reshape(B, 64, 64, C), res


def kernel(x, w_qkv, w_out, b_out):
    out, _ = run(x, w_qkv, w_out, b_out, trace=False)
    return out
